# revision 14
# baseline (speedup 1.0000x reference)
"""KoLeo loss kernel for 8 Trainium2 NeuronCores.

Reference computation (B=16384, D=1024):
    xn  = x / max(||x||_2, 1e-12)          # row L2-normalize
    sim = xn @ xn.T                        # B x B cosine similarity
    max_sim[i] = max_{j != i} sim[i, j]    # nearest neighbor (excl. self)
    out = -mean(log(sqrt(2 - 2*max_sim + 1e-8)))

Sharding: rows of x are split across 8 cores (2048 rows each). Each core
computes its 2048 x 16384 slab of the similarity matrix against all of x
(streamed), takes the row max, and returns the per-row maxima. The cheap
nonlinear epilogue (sqrt/log/mean over 16384 scalars) runs on the host in
float64.

Implementation notes:
  - The host pre-normalizes rows (float64) and pre-transposes to x.T
    [D, B] in fp8e4m3 (or bf16), so the device does no transposes and no
    normalization: the kernel is a pure matmul + diagonal fix + row-max.
    fp8 uses DoubleRow perf mode (256-deep contraction per matmul, 2x).
  - Per-core input is x.T *rotated* so each core's own rows are columns
    0..2047. This makes the self-similarity diagonal land at a fixed,
    core-independent position, keeping the program identical across cores
    (pure SPMD): the slab's diagonal entries sit exactly on the diagonal
    of the leading 2048x2048 block. We subtract 2 there (via a -2*I
    constant) before the row max so the self-match (cos=1) never wins.
  - The leading 4 j-blocks of the rhs stream are slices of the resident
    lhsT tile, so only 28/32 column blocks are DMA'd.
"""

import sys

if "/opt/trn_rl_repo" not in sys.path:
    sys.path.insert(0, "/opt/trn_rl_repo")

import numpy as np
import ml_dtypes

import concourse.bass as bass  # noqa: F401  (import keeps bass registered)
import concourse.mybir as mybir
import concourse.tile as tile
from concourse import bacc
from concourse.bass_utils import run_bass_kernel_spmd

P = 128          # SBUF partitions
NBLK = 512       # similarity column block width (= one PSUM bank of f32)
EPS = 1e-8

B = 16384        # rows of x
D = 1024         # feature dim
N_CORES = 8

# Compute mode: "bf16" (1 cycle/row) or "fp8dr" (fp8e4m3 + DoubleRow,
# 0.5 cycles/row). fp8 inputs are pre-scaled by FP8_SCALE so the unit-norm
# components (sigma ~ 1/32) sit in e4m3's normal range; similarities then
# come out scaled by FP8_SCALE**2, undone on the host.
import os as _os

# fp8dr measured 461,913 ns / rel err 1.6e-3 on HW; bf16 measured
# 901,380 ns / rel err 2.7e-6. Both pass; fp8dr is ~1.95x faster and
# keeps >10x margin to the 2e-2 family accuracy gate.
MODE = _os.environ.get("KOLEO_MODE", "fp8dr")
FP8_SCALE = 8.0


def build_nc(b=B, d=D, n_cores=N_CORES, mode=MODE):
    """Build the per-core SPMD Bass program.

    Inputs :  xt     [d, b]  bf16/fp8e4m3 — rotated, normalized x.T
              negeye [P, P]  f32 — the constant -2*scale^2 * I
    Output :  out    [P, b//n_cores//P] f32 — out[p, m] = scale^2 *
              max_{j != i} sim[i, j] for local row i = m*P + p
    """
    bl = b // n_cores          # local rows per core
    kch = d // P               # contraction chunks
    mch = bl // P              # output row chunks
    nb = b // NBLK             # column blocks
    diag_nb = bl // NBLK       # leading blocks that overlap the diagonal
    assert bl % NBLK == 0 and d % P == 0 and b % NBLK == 0

    if mode == "bf16":
        in_dt = mybir.dt.bfloat16
        kstep = 1                      # K chunks of 128 per matmul
        perf_mode = None
    else:
        in_dt = mybir.dt.float8e4      # e4m3
        kstep = 2                      # DoubleRow: K chunks of 256
        perf_mode = mybir.MatmulPerfMode.DoubleRow
        assert kch % 2 == 0

    nc = bacc.Bacc("TRN2", target_bir_lowering=False, debug=False,
                   num_devices=n_cores)
    xt = nc.dram_tensor("xt", [d, b], in_dt, kind="ExternalInput")
    negeye = nc.dram_tensor("negeye", [P, P], mybir.dt.float32,
                            kind="ExternalInput")
    out = nc.dram_tensor("out", [P, mch], mybir.dt.float32,
                         kind="ExternalOutput")
    xt_ap = xt[:]
    f32 = mybir.dt.float32

    with tile.TileContext(nc) as tc:
        with (
            tc.tile_pool(name="lhs", bufs=1) as lhs_pool,
            tc.tile_pool(name="rhs", bufs=4) as rhs_pool,
            tc.tile_pool(name="psum", bufs=8, space="PSUM") as psum_pool,
            tc.tile_pool(name="stats", bufs=1) as stats_pool,
        ):
            # Alternate DMA issue between two engines so chunk transfers
            # land on different queues and run in parallel.
            dma_eng = [nc.sync, nc.gpsimd]

            # Resident lhsT: this core's rows, K-on-partitions, one tile
            # per kstep group so the first matmul starts after the first
            # group's DMA instead of the whole slab.
            lhs_tiles = [
                lhs_pool.tile([P, kstep, bl], in_dt, name=f"lhs{g}",
                              tag=f"lhs{g}")
                for g in range(kch // kstep)
            ]
            for k in range(kch):
                g, o = divmod(k, kstep)
                dma_eng[k % 2].dma_start(lhs_tiles[g][:, o, :],
                                         xt_ap[k * P:(k + 1) * P, 0:bl])
            eye = stats_pool.tile([P, P], f32, name="eye")
            nc.gpsimd.dma_start(eye[:], negeye[:])

            maxtiles = [
                stats_pool.tile([P, nb], f32, name=f"maxt{m}", tag=f"maxt{m}")
                for m in range(mch)
            ]
            rowmax = stats_pool.tile([P, mch], f32, name="rowmax")

            for jb in range(nb):
                cols = slice(jb * NBLK, (jb + 1) * NBLK)
                if jb < diag_nb:
                    # rhs block is part of the resident lhs tiles
                    rt = None
                else:
                    rt = rhs_pool.tile([P, kch, NBLK], in_dt, name="rt", tag="rt")
                    for k in range(kch):
                        dma_eng[k % 2].dma_start(rt[:, k, :],
                                                 xt_ap[k * P:(k + 1) * P, cols])
                for m in range(mch):
                    ps = psum_pool.tile([P, NBLK], f32, name="ps", tag="ps")
                    for g in range(kch // kstep):
                        k = g * kstep
                        if kstep == 1:
                            rhs = (lhs_tiles[g][:, 0, cols] if rt is None
                                   else rt[:, k, :])
                            lhsT = lhs_tiles[g][:, 0, m * P:(m + 1) * P]
                        else:
                            rhs = (lhs_tiles[g][:, :, cols] if rt is None
                                   else rt[:, k:k + kstep, :])
                            lhsT = lhs_tiles[g][:, :, m * P:(m + 1) * P]
                        nc.tensor.matmul(
                            ps[:],
                            lhsT,
                            rhs,
                            start=(g == 0),
                            stop=(k + kstep == kch),
                            perf_mode=perf_mode,
                        )
                    if jb == (m * P) // NBLK:
                        # self-similarity lives at ps[p, off + p]: add -2*I
                        off = (m * P) % NBLK
                        nc.vector.tensor_add(
                            out=ps[:, off:off + P],
                            in0=ps[:, off:off + P],
                            in1=eye[:],
                        )
                    nc.vector.reduce_max(
                        out=maxtiles[m][:, jb:jb + 1],
                        in_=ps[:],
                        axis=mybir.AxisListType.X,
                        op=mybir.AluOpType.max,
                    )

            for m in range(mch):
                nc.vector.reduce_max(
                    out=rowmax[:, m:m + 1],
                    in_=maxtiles[m][:],
                    axis=mybir.AxisListType.X,
                    op=mybir.AluOpType.max,
                )
            nc.sync.dma_start(out[:], rowmax[:])

    nc.compile()
    return nc


def prepare_inputs(x, b=B, d=D, n_cores=N_CORES, mode=MODE):
    """Host prep: normalize (f64), transpose, cast, per-core rotate."""
    bl = b // n_cores
    xd = np.asarray(x, dtype=np.float64)
    norms = np.sqrt(np.einsum("ij,ij->i", xd, xd))
    np.maximum(norms, 1e-12, out=norms)
    xn = xd / norms[:, None]
    if mode == "bf16":
        scale = 1.0
        xnt = np.ascontiguousarray(xn.T).astype(ml_dtypes.bfloat16)
    else:
        scale = FP8_SCALE
        xnt = np.ascontiguousarray(xn.T * scale).astype(ml_dtypes.float8_e4m3)
    negeye = np.ascontiguousarray(
        (-2.0 * scale * scale) * np.eye(P, dtype=np.float32))
    in_maps = []
    for c in range(n_cores):
        s = c * bl
        rot = np.concatenate([xnt[:, s:], xnt[:, :s]], axis=1) if s else xnt
        in_maps.append({"xt": np.ascontiguousarray(rot), "negeye": negeye})
    return in_maps


def postprocess(results, b=B, n_cores=N_CORES, mode=MODE):
    """Stitch per-core row-max outputs and apply the scalar epilogue."""
    bl = b // n_cores
    inv = 1.0 if mode == "bf16" else 1.0 / (FP8_SCALE * FP8_SCALE)
    maxsim = np.empty(b, dtype=np.float64)
    for c in range(n_cores):
        o = np.asarray(results[c]["out"], dtype=np.float64)  # [P, mch]
        maxsim[c * bl:(c + 1) * bl] = o.T.reshape(-1) * inv  # i = m*P + p
    d2 = 2.0 - 2.0 * maxsim + EPS
    loss = -0.5 * np.mean(np.log(d2))
    return np.array(loss, dtype=np.float32)


_NC_CACHE = {}


def _get_nc():
    key = (B, D, N_CORES, MODE)
    if key not in _NC_CACHE:
        _NC_CACHE[key] = build_nc(*key)
    return _NC_CACHE[key]


def kernel(x, **_ignored):
    nc = _get_nc()
    in_maps = prepare_inputs(x)
    last_exc = None
    for _attempt in range(3):
        try:
            res = run_bass_kernel_spmd(nc, in_maps,
                                       core_ids=list(range(N_CORES)))
            return postprocess(res.results)
        except Exception as exc:  # transient NRT/tunnel hiccups
            last_exc = exc
    raise last_exc


if __name__ == "__main__":
    x = np.random.default_rng(0).standard_normal((B, D), dtype=np.float32)
    print(kernel(x))
